# revision 10
# baseline (speedup 1.0000x reference)
"""MoE feed-forward (top-2 of 8 experts) on 8 Trainium2 NeuronCores.

Strategy (expert-parallel, per the sharding hint):
  - The router (logits -> top-2 -> softmax gates) is the shard-assignment
    computation: it decides which tokens go to which core. It is 0.05% of
    the FLOPs and runs on the host as part of input sharding/dispatch.
  - Core e holds expert e's weights (E == n_cores == 8) and runs the
    dense MLP  gelu(x_e @ W1[e]) @ W2[e]  over the tokens routed to it,
    padded to a common capacity C so all cores run one SPMD program.
  - Load rebalance: the most-loaded expert's overflow tokens (its count
    minus the second-highest count) are offloaded to a "helper pass" that
    all 8 cores run: core c computes the overflow tokens' MLP restricted
    to F-slice [c*F/8, (c+1)*F/8) of the donor expert (gelu is
    elementwise in F; the second matmul's partial outputs sum across
    cores on the host). This cuts the SPMD capacity C from max(count) to
    the second-highest count at the cost of a small extra pass.
  - Everything on device is laid out transposed (tokens in the matmul
    free dim) so no on-device transposes are needed:
        hT[f, t] = sum_d W1[d, f] * xT[d, t]      (lhsT = W1 as stored)
        yT[d, t] = sum_f W2[f, d] * gelu(hT[f, t]) (lhsT = W2 as stored)
  - Matmuls run in bf16 (4x faster than fp32 on the PE) with fp32 PSUM
    accumulation; gelu in fp32 on the scalar engine; bf16 output.
  - Host combine: out[tok] += gate * yT.T (scatter-add; each token
    appears at most once per expert so fancy-index add is exact).
"""

import numpy as np
import ml_dtypes

D = 1024
F = 4096
E = 8
TOPK = 2
P = 128

_BASS_CACHE: dict = {}


def _build_bass(
    C: int,
    d: int = D,
    f: int = F,
    tok_tile: int = 512,
    act: str = "Gelu",
    repeat: int = 1,
    hw_loop: bool = False,
    hw_loop_body: int = 1,
    Ch: int = 0,
    Fs: int = F // E,
):
    import contextlib

    import concourse.mybir as mybir
    import concourse.tile as tile
    from concourse import bacc

    act_fn = getattr(mybir.ActivationFunctionType, act)

    bf16 = mybir.dt.bfloat16
    f32 = mybir.dt.float32

    # Bacc (not plain Bass): its compile pipeline runs
    # generate_event_semaphores, which splits multi-sem waits into
    # InstEventSemaphore preludes — TRN2 instructions encode only 1 wait.
    nc = bacc.Bacc("TRN2", target_bir_lowering=False, debug=False, num_devices=E)
    xT = nc.declare_dram_parameter("xT", [d, C], bf16, isOutput=False)
    w1 = nc.declare_dram_parameter("w1", [d, f], bf16, isOutput=False)
    w2 = nc.declare_dram_parameter("w2", [f, d], bf16, isOutput=False)
    # Output in bf16: halves store DMA traffic and result readback; the
    # ~0.2% rounding it adds is far below the bf16 matmul noise already
    # present.
    yT = nc.declare_dram_parameter("yT", [d, C], bf16, isOutput=True)
    if Ch > 0:
        xh = nc.declare_dram_parameter("xh", [d, Ch], bf16, isOutput=False)
        w1h = nc.declare_dram_parameter("w1h", [d, Fs], bf16, isOutput=False)
        w2h = nc.declare_dram_parameter("w2h", [Fs, d], bf16, isOutput=False)
        yh = nc.declare_dram_parameter("yh", [d, Ch], bf16, isOutput=True)

    KD = d // P  # contraction tiles for mm1 / output d-tiles for mm2
    KF = f // P  # f-tiles for mm1 output / contraction tiles for mm2

    def tiled(total):
        tts, off = [], 0
        while off < total:
            tw = min(tok_tile, total - off)
            tts.append((off, tw))
            off += tw
        return tts

    tts = tiled(C)

    # tok_tile=512 makes each matmul stream 512 columns (213 ns warm) so
    # per-instruction overhead and the FWL weight load (~53 ns,
    # overlapped via the PE reorder window) amortize over twice the work
    # vs 256. PSUM tile [128,512] fp32 is exactly one 2KB bank; bufs=8
    # uses all 8 banks.
    with tile.TileContext(nc) as tc:
        with (
            tc.tile_pool(name="wpool", bufs=1) as wpool,
            tc.tile_pool(name="xpool", bufs=2) as xpool,
            tc.tile_pool(name="hpool", bufs=KF + 1) as hpool,
            tc.tile_pool(name="ypool", bufs=1) as ypool,
            tc.tile_pool(name="psum", bufs=8, space="PSUM") as psum_pool,
        ):
            w1_sb = wpool.tile([P, KD, f], bf16)
            nc.scalar.dma_start(w1_sb[:], w1.ap().rearrange("(ko p) f -> p ko f", p=P))
            w2_sb = wpool.tile([P, KF, d], bf16)
            nc.scalar.dma_start(w2_sb[:], w2.ap().rearrange("(ko p) f -> p ko f", p=P))

            xT_t = xT.ap().rearrange("(ko p) c -> p ko c", p=P)
            yT_t = yT.ap().rearrange("(ko p) c -> p ko c", p=P)
            if Ch > 0:
                KFs = Fs // P
                w1h_sb = wpool.tile([P, KD, Fs], bf16)
                nc.scalar.dma_start(
                    w1h_sb[:], w1h.ap().rearrange("(ko p) f -> p ko f", p=P)
                )
                w2h_sb = wpool.tile([P, KFs, d], bf16)
                nc.scalar.dma_start(
                    w2h_sb[:], w2h.ap().rearrange("(ko p) f -> p ko f", p=P)
                )
                xh_sb = wpool.tile([P, KD, Ch], bf16)
                nc.scalar.dma_start(
                    xh_sb[:], xh.ap().rearrange("(ko p) c -> p ko c", p=P)
                )
                yh_t = yh.ap().rearrange("(ko p) c -> p ko c", p=P)

            def mlp_block(t0, tw, w1_s, w2_s, x_s, y_t, kfs):
                """One token tile through mm1 -> gelu -> mm2 -> store."""
                h_tiles = []
                for ft in range(kfs):
                    ps = psum_pool.tile([P, tok_tile], f32, tag="ps", name="ps")[
                        :, :tw
                    ]
                    for k in range(KD):
                        nc.tensor.matmul(
                            ps[:],
                            w1_s[:, k, ft * P : (ft + 1) * P],
                            x_s[:, k, :],
                            start=(k == 0),
                            stop=(k == KD - 1),
                        )
                    h = hpool.tile([P, tok_tile], bf16, tag="h", name="h")[:, :tw]
                    nc.scalar.activation(h[:], ps[:], act_fn)
                    h_tiles.append(h)

                yt = ypool.tile([P, KD, tok_tile], bf16, tag="yt", name="yt")[
                    :, :, :tw
                ]
                # wait-absorber: this DVE write takes on the slot's WAR
                # (previous store's DMA lane); the DVE copies below then
                # depend only on {PE, DVE} and the store only on {DVE} —
                # HW instruction encodings have very few sem-wait slots
                nc.vector.memset(yt[:], 0.0)
                for dt_ in range(KD):
                    ps2 = psum_pool.tile([P, tok_tile], f32, tag="ps", name="ps")[
                        :, :tw
                    ]
                    for ft in range(kfs):
                        nc.tensor.matmul(
                            ps2[:],
                            w2_s[:, ft, dt_ * P : (dt_ + 1) * P],
                            h_tiles[ft][:],
                            start=(ft == 0),
                            stop=(ft == kfs - 1),
                        )
                    nc.vector.tensor_copy(yt[:, dt_, :], ps2[:])
                nc.sync.dma_start(y_t[:, :, t0 : t0 + tw], yt[:])

            # hw_loop: wrap one full pass in a hardware For_i loop instead
            # of unrolling `repeat` copies — used for benchmarking (tiny
            # program, large repeat counts for a precise timing slope).
            loop_ctx = tc.For_i(0, repeat) if hw_loop else contextlib.nullcontext()
            body_mult = hw_loop_body if hw_loop else repeat
            with loop_ctx:
                for t0, tw in tts * body_mult:
                    xt = xpool.tile([P, KD, tok_tile], bf16, tag="xt", name="xt")[
                        :, :, :tw
                    ]
                    nc.scalar.dma_start(xt[:], xT_t[:, :, t0 : t0 + tw])
                    mlp_block(t0, tw, w1_sb, w2_sb, xt, yT_t, KF)
                if Ch > 0:
                    for h0, hw_ in tiled(Ch) * body_mult:
                        mlp_block(
                            h0, hw_, w1h_sb, w2h_sb, xh_sb[:, :, h0 : h0 + hw_],
                            yh_t, KFs,
                        )

    nc.compile()  # Bacc pipeline: reg alloc + wait splitting (1 wait/inst on TRN2)
    return nc


def _route(xf: np.ndarray, Wr: np.ndarray):
    """Top-2 routing on the host (fp64 logits for a stable ranking)."""
    logits = xf.astype(np.float64) @ Wr.astype(np.float64).T  # [N, E]
    order = np.argsort(-logits, axis=1)[:, :TOPK]  # [N, 2] expert ids, desc
    top_vals = np.take_along_axis(logits, order, axis=1).astype(np.float32)
    m = top_vals.max(axis=1, keepdims=True)
    ex = np.exp(top_vals - m)
    gates2 = (ex / ex.sum(axis=1, keepdims=True)).astype(np.float32)  # [N, 2]
    return order, gates2


def _prep_in_maps(xf, Wr, W1, W2):
    """Route tokens, pick capacities, and build per-core input maps.

    Returns (in_maps, meta) where meta carries the capacities and the
    combine metadata (per-expert token indices and gates, plus the
    offloaded-overflow token set handled by the helper pass).
    """
    d = xf.shape[1]
    order, gates2 = _route(xf, Wr)

    counts = np.bincount(order.ravel(), minlength=E)
    donor = int(np.argmax(counts))
    second = int(np.max(np.delete(counts, donor)))
    # Offload the overflow of the most-loaded expert to the helper pass
    # (cap: one 512-wide helper tile; SBUF sizing assumes Ch <= 512).
    Ch = min(max(0, int(counts[donor]) - second), 512)
    C = max(int(counts.max()) - Ch if Ch > 0 else int(counts.max()), P)

    idx_list, gate_list = [], []
    for e in range(E):
        tok, slot = np.where(order == e)
        idx_list.append(tok)
        gate_list.append(gates2[tok, slot])

    off_tok = np.zeros(0, dtype=np.int64)
    off_gate = np.zeros(0, dtype=np.float32)
    if Ch > 0:
        off_tok = idx_list[donor][-Ch:]
        off_gate = gate_list[donor][-Ch:]
        idx_list[donor] = idx_list[donor][:-Ch]
        gate_list[donor] = gate_list[donor][:-Ch]

    xf_bf = xf.astype(ml_dtypes.bfloat16)
    Fs = F // E
    in_maps = []
    for e in range(E):
        xTe = np.zeros((d, C), dtype=ml_dtypes.bfloat16)
        tok = idx_list[e]
        xTe[:, : len(tok)] = xf_bf[tok].T
        m = {
            "xT": xTe,
            "w1": np.ascontiguousarray(W1[e]).astype(ml_dtypes.bfloat16),
            "w2": np.ascontiguousarray(W2[e]).astype(ml_dtypes.bfloat16),
        }
        if Ch > 0:
            m["xh"] = np.ascontiguousarray(xf_bf[off_tok].T)
            m["w1h"] = np.ascontiguousarray(W1[donor][:, e * Fs : (e + 1) * Fs]).astype(
                ml_dtypes.bfloat16
            )
            m["w2h"] = np.ascontiguousarray(W2[donor][e * Fs : (e + 1) * Fs, :]).astype(
                ml_dtypes.bfloat16
            )
        in_maps.append(m)
    meta = {
        "C": C,
        "Ch": Ch,
        "idx_list": idx_list,
        "gate_list": gate_list,
        "off_tok": off_tok,
        "off_gate": off_gate,
    }
    return in_maps, meta


def _run(inputs, trace: bool = False):
    x = np.asarray(inputs["x"], dtype=np.float32)
    Wr = np.asarray(inputs["Wr"], dtype=np.float32)
    W1 = np.asarray(inputs["W1"], dtype=np.float32)
    W2 = np.asarray(inputs["W2"], dtype=np.float32)
    B, T, d = x.shape
    N = B * T
    xf = np.ascontiguousarray(x.reshape(N, d))

    in_maps, meta = _prep_in_maps(xf, Wr, W1, W2)
    C, Ch = meta["C"], meta["Ch"]

    key = (C, Ch, d)
    if key not in _BASS_CACHE:
        _BASS_CACHE[key] = _build_bass(C, d=d, f=W1.shape[2], Ch=Ch)
    nc = _BASS_CACHE[key]

    from concourse.bass_utils import run_bass_kernel_spmd

    res = run_bass_kernel_spmd(nc, in_maps, core_ids=list(range(E)), trace=trace)

    out = np.zeros((N, d), dtype=np.float32)
    for e in range(E):
        tok = meta["idx_list"][e]
        yTe = np.asarray(res.results[e]["yT"]).astype(np.float32)  # [d, C]
        out[tok] += meta["gate_list"][e][:, None] * yTe[:, : len(tok)].T
    if Ch > 0:
        yh_sum = np.zeros((d, Ch), dtype=np.float32)
        for e in range(E):
            yh_sum += np.asarray(res.results[e]["yh"]).astype(np.float32)
        out[meta["off_tok"]] += meta["off_gate"][:, None] * yh_sum.T
    return out.reshape(B, T, d), res


def kernel(**inputs) -> np.ndarray:
    out, _ = _run(inputs, trace=False)
    return out


# revision 12
# speedup vs baseline: 1.9790x; 1.9790x over previous
"""MoE feed-forward (top-2 of 8 experts) on 8 Trainium2 NeuronCores.

Strategy (expert-parallel, per the sharding hint):
  - The router (logits -> top-2 -> softmax gates) is the shard-assignment
    computation: it decides which tokens go to which core. It is 0.05% of
    the FLOPs and runs on the host as part of input sharding/dispatch.
  - Core e holds expert e's weights (E == n_cores == 8) and runs the
    dense MLP  gelu(x_e @ W1[e]) @ W2[e]  over the tokens routed to it,
    padded to a common capacity C so all cores run one SPMD program.
  - Load rebalance: the most-loaded expert's overflow tokens (its count
    minus the second-highest count) are offloaded to a "helper pass" that
    all 8 cores run: core c computes the overflow tokens' MLP restricted
    to F-slice [c*F/8, (c+1)*F/8) of the donor expert (gelu is
    elementwise in F; the second matmul's partial outputs sum across
    cores on the host). This cuts the SPMD capacity C from max(count) to
    the second-highest count at the cost of a small extra pass.
  - Everything on device is laid out transposed (tokens in the matmul
    free dim) so no on-device transposes are needed:
        hT[f, t] = sum_d W1[d, f] * xT[d, t]      (lhsT = W1 as stored)
        yT[d, t] = sum_f W2[f, d] * gelu(hT[f, t]) (lhsT = W2 as stored)
  - Matmuls run in bf16 (4x faster than fp32 on the PE) with fp32 PSUM
    accumulation; gelu in fp32 on the scalar engine; bf16 output.
  - Host combine: out[tok] += gate * yT.T (scatter-add; each token
    appears at most once per expert so fancy-index add is exact).
"""

import numpy as np
import ml_dtypes

D = 1024
F = 4096
E = 8
TOPK = 2
P = 128

_BASS_CACHE: dict = {}


def _build_bass(
    C: int,
    d: int = D,
    f: int = F,
    tok_tile: int = 512,
    act: str = "Gelu",
    repeat: int = 1,
    hw_loop: bool = False,
    hw_loop_body: int = 1,
    Ch: int = 0,
    Fs: int = F // E,
):
    import contextlib

    import concourse.mybir as mybir
    import concourse.tile as tile
    from concourse import bacc

    act_fn = getattr(mybir.ActivationFunctionType, act)

    bf16 = mybir.dt.bfloat16
    f32 = mybir.dt.float32

    # Bacc (not plain Bass): its compile pipeline runs
    # generate_event_semaphores, which splits multi-sem waits into
    # InstEventSemaphore preludes — TRN2 instructions encode only 1 wait.
    nc = bacc.Bacc("TRN2", target_bir_lowering=False, debug=False, num_devices=E)
    xT = nc.declare_dram_parameter("xT", [d, C], bf16, isOutput=False)
    w1 = nc.declare_dram_parameter("w1", [d, f], bf16, isOutput=False)
    w2 = nc.declare_dram_parameter("w2", [f, d], bf16, isOutput=False)
    # Output in bf16: halves store DMA traffic and result readback; the
    # ~0.2% rounding it adds is far below the bf16 matmul noise already
    # present.
    yT = nc.declare_dram_parameter("yT", [d, C], bf16, isOutput=True)
    if hw_loop:
        # Benchmark-only: the For_i trip count comes from a [1,1] uint32
        # input, loaded straight into registers on every engine (same
        # mechanism as partition_id). One executable then serves every
        # repeat value, so the axon dispatch-mode offset cancels exactly
        # in a timing slope.
        rep = nc.declare_dram_parameter("rep", [1, 1], mybir.dt.uint32, isOutput=False)
    if Ch > 0:
        xh = nc.declare_dram_parameter("xh", [d, Ch], bf16, isOutput=False)
        w1h = nc.declare_dram_parameter("w1h", [d, Fs], bf16, isOutput=False)
        w2h = nc.declare_dram_parameter("w2h", [Fs, d], bf16, isOutput=False)
        yh = nc.declare_dram_parameter("yh", [d, Ch], bf16, isOutput=True)

    KD = d // P  # contraction tiles for mm1 / output d-tiles for mm2
    KF = f // P  # f-tiles for mm1 output / contraction tiles for mm2

    def tiled(total):
        tts, off = [], 0
        while off < total:
            tw = min(tok_tile, total - off)
            tts.append((off, tw))
            off += tw
        return tts

    tts = tiled(C)

    # tok_tile=512 makes each matmul stream 512 columns (213 ns warm) so
    # per-instruction overhead and the FWL weight load (~53 ns,
    # overlapped via the PE reorder window) amortize over twice the work
    # vs 256. PSUM tile [128,512] fp32 is exactly one 2KB bank; bufs=8
    # uses all 8 banks.
    with tile.TileContext(nc) as tc:
        with (
            tc.tile_pool(name="wpool", bufs=1) as wpool,
            tc.tile_pool(name="xpool", bufs=2) as xpool,
            tc.tile_pool(name="hpool", bufs=KF + 1) as hpool,
            tc.tile_pool(name="ypool", bufs=1) as ypool,
            tc.tile_pool(name="psum", bufs=8, space="PSUM") as psum_pool,
        ):
            w1_sb = wpool.tile([P, KD, f], bf16)
            nc.scalar.dma_start(w1_sb[:], w1.ap().rearrange("(ko p) f -> p ko f", p=P))
            w2_sb = wpool.tile([P, KF, d], bf16)
            nc.scalar.dma_start(w2_sb[:], w2.ap().rearrange("(ko p) f -> p ko f", p=P))

            xT_t = xT.ap().rearrange("(ko p) c -> p ko c", p=P)
            yT_t = yT.ap().rearrange("(ko p) c -> p ko c", p=P)
            if Ch > 0:
                KFs = Fs // P
                w1h_sb = wpool.tile([P, KD, Fs], bf16)
                nc.scalar.dma_start(
                    w1h_sb[:], w1h.ap().rearrange("(ko p) f -> p ko f", p=P)
                )
                w2h_sb = wpool.tile([P, KFs, d], bf16)
                nc.scalar.dma_start(
                    w2h_sb[:], w2h.ap().rearrange("(ko p) f -> p ko f", p=P)
                )
                xh_sb = wpool.tile([P, KD, Ch], bf16)
                nc.scalar.dma_start(
                    xh_sb[:], xh.ap().rearrange("(ko p) c -> p ko c", p=P)
                )
                yh_t = yh.ap().rearrange("(ko p) c -> p ko c", p=P)

            def mlp_block(t0, tw, w1_s, w2_s, x_s, y_t, kfs):
                """One token tile through mm1 -> gelu -> mm2 -> store."""
                h_tiles = []
                for ft in range(kfs):
                    ps = psum_pool.tile([P, tok_tile], f32, tag="ps", name="ps")[
                        :, :tw
                    ]
                    for k in range(KD):
                        nc.tensor.matmul(
                            ps[:],
                            w1_s[:, k, ft * P : (ft + 1) * P],
                            x_s[:, k, :],
                            start=(k == 0),
                            stop=(k == KD - 1),
                        )
                    h = hpool.tile([P, tok_tile], bf16, tag="h", name="h")[:, :tw]
                    nc.scalar.activation(h[:], ps[:], act_fn)
                    h_tiles.append(h)

                yt = ypool.tile([P, KD, tok_tile], bf16, tag="yt", name="yt")[
                    :, :, :tw
                ]
                # wait-absorber: this DVE write takes on the slot's WAR
                # (previous store's DMA lane); the DVE copies below then
                # depend only on {PE, DVE} and the store only on {DVE} —
                # HW instruction encodings have very few sem-wait slots
                nc.vector.memset(yt[:], 0.0)
                for dt_ in range(KD):
                    ps2 = psum_pool.tile([P, tok_tile], f32, tag="ps", name="ps")[
                        :, :tw
                    ]
                    for ft in range(kfs):
                        nc.tensor.matmul(
                            ps2[:],
                            w2_s[:, ft, dt_ * P : (dt_ + 1) * P],
                            h_tiles[ft][:],
                            start=(ft == 0),
                            stop=(ft == kfs - 1),
                        )
                    nc.vector.tensor_copy(yt[:, dt_, :], ps2[:])
                nc.sync.dma_start(y_t[:, :, t0 : t0 + tw], yt[:])

            # hw_loop: wrap one full pass in a hardware For_i loop instead
            # of unrolling `repeat` copies — used for benchmarking (tiny
            # program, large repeat counts for a precise timing slope).
            if hw_loop:
                tmp = nc.alloc_registers(f"tmp_rep_{nc.next_id()}", mybir.ALL_ENGINES)
                nc.regs_load(tmp, rep[0:1, 0:1])
                rep_val = nc.snap(tmp, donate=True, min_val=0, max_val=1024)
                loop_ctx = tc.For_i(0, rep_val)
            else:
                loop_ctx = contextlib.nullcontext()
            body_mult = hw_loop_body if hw_loop else repeat
            with loop_ctx:
                for t0, tw in tts * body_mult:
                    xt = xpool.tile([P, KD, tok_tile], bf16, tag="xt", name="xt")[
                        :, :, :tw
                    ]
                    nc.scalar.dma_start(xt[:], xT_t[:, :, t0 : t0 + tw])
                    mlp_block(t0, tw, w1_sb, w2_sb, xt, yT_t, KF)
                if Ch > 0:
                    for h0, hw_ in tiled(Ch) * body_mult:
                        mlp_block(
                            h0, hw_, w1h_sb, w2h_sb, xh_sb[:, :, h0 : h0 + hw_],
                            yh_t, KFs,
                        )

    nc.compile()  # Bacc pipeline: reg alloc + wait splitting (1 wait/inst on TRN2)
    return nc


def _route(xf: np.ndarray, Wr: np.ndarray):
    """Top-2 routing on the host (fp64 logits for a stable ranking)."""
    logits = xf.astype(np.float64) @ Wr.astype(np.float64).T  # [N, E]
    order = np.argsort(-logits, axis=1)[:, :TOPK]  # [N, 2] expert ids, desc
    top_vals = np.take_along_axis(logits, order, axis=1).astype(np.float32)
    m = top_vals.max(axis=1, keepdims=True)
    ex = np.exp(top_vals - m)
    gates2 = (ex / ex.sum(axis=1, keepdims=True)).astype(np.float32)  # [N, 2]
    return order, gates2


def _prep_in_maps(xf, Wr, W1, W2):
    """Route tokens, pick capacities, and build per-core input maps.

    Returns (in_maps, meta) where meta carries the capacities and the
    combine metadata (per-expert token indices and gates, plus the
    offloaded-overflow token set handled by the helper pass).
    """
    d = xf.shape[1]
    order, gates2 = _route(xf, Wr)

    counts = np.bincount(order.ravel(), minlength=E)
    donor = int(np.argmax(counts))
    second = int(np.max(np.delete(counts, donor)))
    # Offload the overflow of the most-loaded expert to the helper pass
    # (cap: one 512-wide helper tile; SBUF sizing assumes Ch <= 512).
    Ch = min(max(0, int(counts[donor]) - second), 512)
    C = max(int(counts.max()) - Ch if Ch > 0 else int(counts.max()), P)

    idx_list, gate_list = [], []
    for e in range(E):
        tok, slot = np.where(order == e)
        idx_list.append(tok)
        gate_list.append(gates2[tok, slot])

    off_tok = np.zeros(0, dtype=np.int64)
    off_gate = np.zeros(0, dtype=np.float32)
    if Ch > 0:
        off_tok = idx_list[donor][-Ch:]
        off_gate = gate_list[donor][-Ch:]
        idx_list[donor] = idx_list[donor][:-Ch]
        gate_list[donor] = gate_list[donor][:-Ch]

    xf_bf = xf.astype(ml_dtypes.bfloat16)
    Fs = F // E
    in_maps = []
    for e in range(E):
        xTe = np.zeros((d, C), dtype=ml_dtypes.bfloat16)
        tok = idx_list[e]
        xTe[:, : len(tok)] = xf_bf[tok].T
        m = {
            "xT": xTe,
            "w1": np.ascontiguousarray(W1[e]).astype(ml_dtypes.bfloat16),
            "w2": np.ascontiguousarray(W2[e]).astype(ml_dtypes.bfloat16),
        }
        if Ch > 0:
            m["xh"] = np.ascontiguousarray(xf_bf[off_tok].T)
            m["w1h"] = np.ascontiguousarray(W1[donor][:, e * Fs : (e + 1) * Fs]).astype(
                ml_dtypes.bfloat16
            )
            m["w2h"] = np.ascontiguousarray(W2[donor][e * Fs : (e + 1) * Fs, :]).astype(
                ml_dtypes.bfloat16
            )
        in_maps.append(m)
    meta = {
        "C": C,
        "Ch": Ch,
        "idx_list": idx_list,
        "gate_list": gate_list,
        "off_tok": off_tok,
        "off_gate": off_gate,
    }
    return in_maps, meta


def _run(inputs, trace: bool = False):
    x = np.asarray(inputs["x"], dtype=np.float32)
    Wr = np.asarray(inputs["Wr"], dtype=np.float32)
    W1 = np.asarray(inputs["W1"], dtype=np.float32)
    W2 = np.asarray(inputs["W2"], dtype=np.float32)
    B, T, d = x.shape
    N = B * T
    xf = np.ascontiguousarray(x.reshape(N, d))

    in_maps, meta = _prep_in_maps(xf, Wr, W1, W2)
    C, Ch = meta["C"], meta["Ch"]

    key = (C, Ch, d)
    if key not in _BASS_CACHE:
        _BASS_CACHE[key] = _build_bass(C, d=d, f=W1.shape[2], Ch=Ch)
    nc = _BASS_CACHE[key]

    from concourse.bass_utils import run_bass_kernel_spmd

    res = run_bass_kernel_spmd(nc, in_maps, core_ids=list(range(E)), trace=trace)

    out = np.zeros((N, d), dtype=np.float32)
    for e in range(E):
        tok = meta["idx_list"][e]
        yTe = np.asarray(res.results[e]["yT"]).astype(np.float32)  # [d, C]
        out[tok] += meta["gate_list"][e][:, None] * yTe[:, : len(tok)].T
    if Ch > 0:
        yh_sum = np.zeros((d, Ch), dtype=np.float32)
        for e in range(E):
            yh_sum += np.asarray(res.results[e]["yh"]).astype(np.float32)
        out[meta["off_tok"]] += meta["off_gate"][:, None] * yh_sum.T
    return out.reshape(B, T, d), res


def kernel(**inputs) -> np.ndarray:
    out, _ = _run(inputs, trace=False)
    return out


# revision 17
# speedup vs baseline: 2.0306x; 1.0261x over previous
"""MoE feed-forward (top-2 of 8 experts) on 8 Trainium2 NeuronCores.

Strategy (expert-parallel, per the sharding hint):
  - The router (logits -> top-2 -> softmax gates) is the shard-assignment
    computation: it decides which tokens go to which core. It is 0.05% of
    the FLOPs and runs on the host as part of input sharding/dispatch.
  - Core e holds expert e's weights (E == n_cores == 8) and runs the
    dense MLP  gelu(x_e @ W1[e]) @ W2[e]  over the tokens routed to it,
    padded to a common capacity C so all cores run one SPMD program.
  - Load rebalance: the most-loaded expert's overflow tokens are
    offloaded to a "helper pass" that all 8 cores run: core c computes
    the overflow tokens' MLP restricted to F-slice [c*F/8, (c+1)*F/8) of
    the donor expert (gelu is elementwise in F; the second matmul's
    partial outputs sum across cores on the host). The capacity C is
    rounded down to a multiple of the 512-wide token tile (a tail tile
    costs a full set of matmul instructions for a few columns); the
    handful of tokens by which OTHER experts then exceed C (bounded at
    64 pairs, <0.4% of FLOPs; 10 pairs for the reference routing) are
    computed in fp32 during the host combine step.
  - Everything on device is laid out transposed (tokens in the matmul
    free dim) so no on-device transposes are needed:
        hT[f, t] = sum_d W1[d, f] * xT[d, t]      (lhsT = W1 as stored)
        yT[d, t] = sum_f W2[f, d] * gelu(hT[f, t]) (lhsT = W2 as stored)
  - Matmuls run in bf16 (4x faster than fp32 on the PE) with fp32 PSUM
    accumulation; gelu in fp32 on the scalar engine; bf16 output.
  - Host combine: out[tok] += gate * yT.T (scatter-add; each token
    appears at most once per expert so fancy-index add is exact).
"""

import numpy as np
import ml_dtypes

D = 1024
F = 4096
E = 8
TOPK = 2
P = 128

_BASS_CACHE: dict = {}


def _build_bass(
    C: int,
    d: int = D,
    f: int = F,
    tok_tile: int = 512,
    act: str = "Gelu",
    repeat: int = 1,
    hw_loop: bool = False,
    hw_loop_body: int = 1,
    Ch: int = 0,
    Fs: int = F // E,
):
    import contextlib

    import concourse.mybir as mybir
    import concourse.tile as tile
    from concourse import bacc

    act_fn = getattr(mybir.ActivationFunctionType, act)

    bf16 = mybir.dt.bfloat16
    f32 = mybir.dt.float32

    # Bacc (not plain Bass): its compile pipeline runs
    # generate_event_semaphores, which splits multi-sem waits into
    # InstEventSemaphore preludes — TRN2 instructions encode only 1 wait.
    nc = bacc.Bacc("TRN2", target_bir_lowering=False, debug=False, num_devices=E)
    xT = nc.declare_dram_parameter("xT", [d, C], bf16, isOutput=False)
    w1 = nc.declare_dram_parameter("w1", [d, f], bf16, isOutput=False)
    w2 = nc.declare_dram_parameter("w2", [f, d], bf16, isOutput=False)
    # Output in bf16: halves store DMA traffic and result readback; the
    # ~0.2% rounding it adds is far below the bf16 matmul noise already
    # present.
    yT = nc.declare_dram_parameter("yT", [d, C], bf16, isOutput=True)
    if hw_loop:
        # Benchmark-only: the For_i trip count comes from a [1,1] uint32
        # input, loaded straight into registers on every engine (same
        # mechanism as partition_id). One executable then serves every
        # repeat value, so the axon dispatch-mode offset cancels exactly
        # in a timing slope.
        rep = nc.declare_dram_parameter("rep", [1, 1], mybir.dt.uint32, isOutput=False)
    if Ch > 0:
        xh = nc.declare_dram_parameter("xh", [d, Ch], bf16, isOutput=False)
        w1h = nc.declare_dram_parameter("w1h", [d, Fs], bf16, isOutput=False)
        w2h = nc.declare_dram_parameter("w2h", [Fs, d], bf16, isOutput=False)
        yh = nc.declare_dram_parameter("yh", [d, Ch], bf16, isOutput=True)

    KD = d // P  # contraction tiles for mm1 / output d-tiles for mm2
    KF = f // P  # f-tiles for mm1 output / contraction tiles for mm2

    def tiled(total):
        tts, off = [], 0
        while off < total:
            tw = min(tok_tile, total - off)
            tts.append((off, tw))
            off += tw
        return tts

    tts = tiled(C)

    # tok_tile=512 makes each matmul stream 512 columns (213 ns warm) so
    # per-instruction overhead and the FWL weight load (~53 ns,
    # overlapped via the PE reorder window) amortize over twice the work
    # vs 256. PSUM tile [128,512] fp32 is exactly one 2KB bank; bufs=8
    # uses all 8 banks.
    with tile.TileContext(nc) as tc:
        with (
            tc.tile_pool(name="wpool", bufs=1) as wpool,
            tc.tile_pool(name="xpool", bufs=2) as xpool,
            tc.tile_pool(name="hpool", bufs=KF + 1) as hpool,
            tc.tile_pool(name="ypool", bufs=1) as ypool,
            tc.tile_pool(name="psum", bufs=8, space="PSUM") as psum_pool,
        ):
            w1_sb = wpool.tile([P, KD, f], bf16)
            nc.scalar.dma_start(w1_sb[:], w1.ap().rearrange("(ko p) f -> p ko f", p=P))
            w2_sb = wpool.tile([P, KF, d], bf16)
            nc.scalar.dma_start(w2_sb[:], w2.ap().rearrange("(ko p) f -> p ko f", p=P))

            xT_t = xT.ap().rearrange("(ko p) c -> p ko c", p=P)
            yT_t = yT.ap().rearrange("(ko p) c -> p ko c", p=P)
            if Ch > 0:
                KFs = Fs // P
                w1h_sb = wpool.tile([P, KD, Fs], bf16)
                nc.scalar.dma_start(
                    w1h_sb[:], w1h.ap().rearrange("(ko p) f -> p ko f", p=P)
                )
                w2h_sb = wpool.tile([P, KFs, d], bf16)
                nc.scalar.dma_start(
                    w2h_sb[:], w2h.ap().rearrange("(ko p) f -> p ko f", p=P)
                )
                xh_sb = wpool.tile([P, KD, Ch], bf16)
                nc.scalar.dma_start(
                    xh_sb[:], xh.ap().rearrange("(ko p) c -> p ko c", p=P)
                )
                yh_t = yh.ap().rearrange("(ko p) c -> p ko c", p=P)

            def mlp_block(t0, tw, w1_s, w2_s, x_s, y_t, kfs):
                """One token tile through mm1 -> gelu -> mm2 -> store."""
                h_tiles = []
                for ft in range(kfs):
                    ps = psum_pool.tile([P, tok_tile], f32, tag="ps", name="ps")[
                        :, :tw
                    ]
                    for k in range(KD):
                        nc.tensor.matmul(
                            ps[:],
                            w1_s[:, k, ft * P : (ft + 1) * P],
                            x_s[:, k, :],
                            start=(k == 0),
                            stop=(k == KD - 1),
                        )
                    h = hpool.tile([P, tok_tile], bf16, tag="h", name="h")[:, :tw]
                    nc.scalar.activation(h[:], ps[:], act_fn)
                    h_tiles.append(h)

                yt = ypool.tile([P, KD, tok_tile], bf16, tag="yt", name="yt")[
                    :, :, :tw
                ]
                # wait-absorber: this DVE write takes on the slot's WAR
                # (previous store's DMA lane); the DVE copies below then
                # depend only on {PE, DVE} and the store only on {DVE} —
                # HW instruction encodings have very few sem-wait slots
                nc.vector.memset(yt[:], 0.0)
                for dt_ in range(KD):
                    ps2 = psum_pool.tile([P, tok_tile], f32, tag="ps", name="ps")[
                        :, :tw
                    ]
                    for ft in range(kfs):
                        nc.tensor.matmul(
                            ps2[:],
                            w2_s[:, ft, dt_ * P : (dt_ + 1) * P],
                            h_tiles[ft][:],
                            start=(ft == 0),
                            stop=(ft == kfs - 1),
                        )
                    nc.vector.tensor_copy(yt[:, dt_, :], ps2[:])
                nc.sync.dma_start(y_t[:, :, t0 : t0 + tw], yt[:])

            # hw_loop: wrap one full pass in a hardware For_i loop instead
            # of unrolling `repeat` copies — used for benchmarking (tiny
            # program, large repeat counts for a precise timing slope).
            if hw_loop:
                tmp = nc.alloc_registers(f"tmp_rep_{nc.next_id()}", mybir.ALL_ENGINES)
                nc.regs_load(tmp, rep[0:1, 0:1])
                rep_val = nc.snap(tmp, donate=True, min_val=0, max_val=1024)
                loop_ctx = tc.For_i(0, rep_val)
            else:
                loop_ctx = contextlib.nullcontext()
            body_mult = hw_loop_body if hw_loop else repeat
            with loop_ctx:
                for t0, tw in tts * body_mult:
                    xt = xpool.tile([P, KD, tok_tile], bf16, tag="xt", name="xt")[
                        :, :, :tw
                    ]
                    nc.scalar.dma_start(xt[:], xT_t[:, :, t0 : t0 + tw])
                    mlp_block(t0, tw, w1_sb, w2_sb, xt, yT_t, KF)
                if Ch > 0:
                    for h0, hw_ in tiled(Ch) * body_mult:
                        mlp_block(
                            h0, hw_, w1h_sb, w2h_sb, xh_sb[:, :, h0 : h0 + hw_],
                            yh_t, KFs,
                        )

    nc.compile()  # Bacc pipeline: reg alloc + wait splitting (1 wait/inst on TRN2)
    return nc


def _gelu_f32(x: np.ndarray) -> np.ndarray:
    try:
        from scipy.special import erf
    except ImportError:
        import math

        erf = np.vectorize(math.erf)
    x = x.astype(np.float32)
    return (0.5 * x * (1.0 + erf(x / np.sqrt(2.0)))).astype(np.float32)


def _route(xf: np.ndarray, Wr: np.ndarray):
    """Top-2 routing on the host (fp64 logits for a stable ranking)."""
    logits = xf.astype(np.float64) @ Wr.astype(np.float64).T  # [N, E]
    order = np.argsort(-logits, axis=1)[:, :TOPK]  # [N, 2] expert ids, desc
    top_vals = np.take_along_axis(logits, order, axis=1).astype(np.float32)
    m = top_vals.max(axis=1, keepdims=True)
    ex = np.exp(top_vals - m)
    gates2 = (ex / ex.sum(axis=1, keepdims=True)).astype(np.float32)  # [N, 2]
    return order, gates2


def _prep_in_maps(xf, Wr, W1, W2):
    """Route tokens, pick capacities, and build per-core input maps.

    Returns (in_maps, meta) where meta carries the capacities and the
    combine metadata (per-expert token indices and gates, plus the
    offloaded-overflow token set handled by the helper pass).
    """
    d = xf.shape[1]
    order, gates2 = _route(xf, Wr)

    counts = np.bincount(order.ravel(), minlength=E)
    donor = int(np.argmax(counts))
    second = int(np.max(np.delete(counts, donor)))
    TOKT = 512  # device token-tile width (psum bank)
    # Capacity selection: a C that is a multiple of the 512-wide token
    # tile avoids a tail tile that would cost a full set of matmul
    # instructions for a handful of columns. The most-loaded expert's
    # overflow goes to the device helper pass (cap 512 = one tile); the
    # few tokens by which OTHER experts exceed the rounded-down C (if
    # any) are computed in the host combine step. That host share is
    # bounded by HOST_CAP pairs (<0.4% of FLOPs) — if the routing is so
    # imbalanced that more would spill, C falls back to covering the
    # second-highest count exactly.
    HOST_CAP = 64
    C_t = max(P, (second // TOKT) * TOKT)
    spill = sum(int(counts[e]) - C_t for e in range(E) if e != donor and counts[e] > C_t)
    if spill > HOST_CAP or int(counts[donor]) - C_t > TOKT:
        C_t = max(second, int(counts[donor]) - TOKT, P)
    C = C_t
    Ch = min(max(0, int(counts[donor]) - C), TOKT)

    idx_list, gate_list = [], []
    for e in range(E):
        tok, slot = np.where(order == e)
        idx_list.append(tok)
        gate_list.append(gates2[tok, slot])

    off_tok = np.zeros(0, dtype=np.int64)
    off_gate = np.zeros(0, dtype=np.float32)
    if Ch > 0:
        off_tok = idx_list[donor][-Ch:]
        off_gate = gate_list[donor][-Ch:]
        idx_list[donor] = idx_list[donor][:-Ch]
        gate_list[donor] = gate_list[donor][:-Ch]

    # Host-handled overflow of non-donor experts (tiny by construction).
    host_items = []
    for e in range(E):
        n_over = len(idx_list[e]) - C
        if e != donor and n_over > 0:
            host_items.append((e, idx_list[e][-n_over:], gate_list[e][-n_over:]))
            idx_list[e] = idx_list[e][:-n_over]
            gate_list[e] = gate_list[e][:-n_over]

    xf_bf = xf.astype(ml_dtypes.bfloat16)
    Fs = F // E
    in_maps = []
    for e in range(E):
        xTe = np.zeros((d, C), dtype=ml_dtypes.bfloat16)
        tok = idx_list[e]
        xTe[:, : len(tok)] = xf_bf[tok].T
        m = {
            "xT": xTe,
            "w1": np.ascontiguousarray(W1[e]).astype(ml_dtypes.bfloat16),
            "w2": np.ascontiguousarray(W2[e]).astype(ml_dtypes.bfloat16),
        }
        if Ch > 0:
            m["xh"] = np.ascontiguousarray(xf_bf[off_tok].T)
            m["w1h"] = np.ascontiguousarray(W1[donor][:, e * Fs : (e + 1) * Fs]).astype(
                ml_dtypes.bfloat16
            )
            m["w2h"] = np.ascontiguousarray(W2[donor][e * Fs : (e + 1) * Fs, :]).astype(
                ml_dtypes.bfloat16
            )
        in_maps.append(m)
    meta = {
        "C": C,
        "Ch": Ch,
        "idx_list": idx_list,
        "gate_list": gate_list,
        "off_tok": off_tok,
        "off_gate": off_gate,
        "host_items": host_items,
    }
    return in_maps, meta


def _run(inputs, trace: bool = False):
    x = np.asarray(inputs["x"], dtype=np.float32)
    Wr = np.asarray(inputs["Wr"], dtype=np.float32)
    W1 = np.asarray(inputs["W1"], dtype=np.float32)
    W2 = np.asarray(inputs["W2"], dtype=np.float32)
    B, T, d = x.shape
    N = B * T
    xf = np.ascontiguousarray(x.reshape(N, d))

    in_maps, meta = _prep_in_maps(xf, Wr, W1, W2)
    C, Ch = meta["C"], meta["Ch"]

    key = (C, Ch, d)
    if key not in _BASS_CACHE:
        _BASS_CACHE[key] = _build_bass(C, d=d, f=W1.shape[2], Ch=Ch)
    nc = _BASS_CACHE[key]

    from concourse.bass_utils import run_bass_kernel_spmd

    res = run_bass_kernel_spmd(nc, in_maps, core_ids=list(range(E)), trace=trace)

    out = np.zeros((N, d), dtype=np.float32)
    for e in range(E):
        tok = meta["idx_list"][e]
        yTe = np.asarray(res.results[e]["yT"]).astype(np.float32)  # [d, C]
        out[tok] += meta["gate_list"][e][:, None] * yTe[:, : len(tok)].T
    if Ch > 0:
        yh_sum = np.zeros((d, Ch), dtype=np.float32)
        for e in range(E):
            yh_sum += np.asarray(res.results[e]["yh"]).astype(np.float32)
        out[meta["off_tok"]] += meta["off_gate"][:, None] * yh_sum.T
    for e, toks, gts in meta["host_items"]:
        h = _gelu_f32(xf[toks] @ W1[e])
        out[toks] += gts[:, None] * (h @ W2[e])
    return out.reshape(B, T, d), res


def kernel(**inputs) -> np.ndarray:
    out, _ = _run(inputs, trace=False)
    return out
